# revision 29
# baseline (speedup 1.0000x reference)
"""Distributed causal RoPE attention for Trainium2 (8 NeuronCores).

Mesh: 2 (batch) x 4 (head-group tensor-parallel).
Core c = b*4 + g handles batch b, heads [4g, 4g+4).

v2 design (vs. v1 baseline):
  - bf16 operands on the PE everywhere (PSUM accumulation stays fp32);
    fp32 error headroom is ~50x the tolerance, bf16 lands well inside it.
  - Fused single-pass QKV: one xT stream, each x tile stationary for
    three matmuls (Q/K/V) -> half the HBM traffic of the two-pass v1.
  - Scores computed TRANSPOSED: S^T[k, q] = (K tile).T stationary @ Q^T
    streaming, exp on ScalarE writes A^T tiles directly -> the 544 PE
    transposes + DVE casts of attn tiles in v1 are gone entirely.
  - PV: lhsT = A^T tile (stationary), rhs = [V | ones] so the PSUM
    output is [O | rowsum] -- softmax denominator comes out of the
    matmul for free as column 128, normalized with a cheap per-partition
    scalar multiply.
  - Projection partials + ReduceScatter in bf16 (CCE supports bf16):
    halves the collective time; host casts the bf16 output to fp32.
  - Host pre-lays-out every input so each tensor is ONE contiguous DMA.
"""

import sys

sys.path.insert(0, "/opt/trn_rl_repo")

import numpy as np

import concourse.bass as bass
import concourse.mybir as mybir
import concourse.tile as tile
from concourse.bass_utils import run_bass_kernel_spmd
from concourse.tile import add_dep_helper
from concourse.masks import make_identity

FP = mybir.dt.float32
BF = mybir.dt.bfloat16
D = 2048  # d_model
S = 2048  # sequence length
B = 2  # batch
NH = 16  # heads
DKV = 128  # head dim
THETA = 10000.0
TP = 4  # head-parallel groups
HPC = NH // TP  # heads per core = 4
HD = HPC * DKV  # head dims per core = 512
NQT = S // 128  # 16 query tiles
NDC = D // 128  # 16 contraction chunks
QB = 4  # q-tiles per attention block
NB = NQT // QB  # 4 blocks
SCALE = 1.0 / float(np.sqrt(DKV))
N_CORES = 8

# attention blocks are computed in order 0,1,3,2 so the reduce-scatter
# pipeline drains early; each chunk is a contiguous run of q-tiles and
# fires as soon as its last q-tile (in compute order) is projected.  The
# final chunk is a single q-tile so the end-of-kernel collective is tiny.
BLOCK_ORDER = [0, 1, 2, 3]
CHUNK_QTS = [[0, 1, 2, 3], [4, 5, 6, 7], [8, 9, 10, 11], [12, 13, 14], [15]]
RS_CHUNKS = [len(q) for q in CHUNK_QTS]


def _legalize_waits(nc):
    """This walrus build only accepts one embedded sync-wait per TPB
    instruction ("Too many sync wait commands").  Split excess waits of
    compute-engine instructions into preceding engine-local NoOps, each
    carrying a single wait.  DMA (queue-embedded) waits are left alone.
    """
    n_split = 0
    for f in nc.m.functions:
        for bb in f.blocks:
            out = []
            for ins in bb.instructions:
                si = ins.sync_info
                if (
                    si is not None
                    and len(si.on_wait) > 1
                    and ins.engine != mybir.EngineType.Unassigned
                ):
                    waits = {}
                    for w in si.on_wait:
                        key = (w.sync_type, w.id, w.wait_mode)
                        if key not in waits or (
                            w.wait_value is not None
                            and waits[key].wait_value is not None
                            and w.wait_value > waits[key].wait_value
                        ):
                            waits[key] = w
                    waits = list(waits.values())
                    for w in waits[:-1]:
                        nop = mybir.InstNoOp(name=f"{ins.name}-waitsplit-{n_split}")
                        n_split += 1
                        nop.engine = ins.engine
                        nop.sync_info = mybir.SyncInfo(on_wait=[w], on_update=[])
                        out.append(nop)
                    ins.sync_info = mybir.SyncInfo(
                        on_wait=[waits[-1]], on_update=si.on_update
                    )
                out.append(ins)
            bb.instructions = out
    return n_split


def build_nc():
    nc = bass.Bass()

    # Host-pre-transposed, bf16, each a single contiguous DMA.
    xT = nc.declare_dram_parameter("xT", [NQT, 128, NDC * 128], BF, isOutput=False)
    wq = nc.declare_dram_parameter("wq", [128, NDC * HD], BF, isOutput=False)
    wk = nc.declare_dram_parameter("wk", [128, NDC * HD], BF, isOutput=False)
    wv = nc.declare_dram_parameter("wv", [128, NDC * HD], BF, isOutput=False)
    wo = nc.declare_dram_parameter("wo", [128, HPC * D], BF, isOutput=False)
    cosp = nc.declare_dram_parameter("cosp", [128, NQT * 64], FP, isOutput=False)
    sinp = nc.declare_dram_parameter("sinp", [128, NQT * 64], FP, isOutput=False)
    out = nc.declare_dram_parameter("out", [S // TP, D], BF, isOutput=True)

    with tile.TileContext(nc) as tc:
        with (
            tc.tile_pool(name="dram", bufs=1, space="DRAM") as dram,
            tc.tile_pool(name="const", bufs=1) as constp,
            tc.tile_pool(name="resident", bufs=1) as resp,
        ):
            partials = [
                dram.tile([n * 128, D], BF, name=f"partial{c}", tag=f"partial{c}")
                for c, n in enumerate(RS_CHUNKS)
            ]
            rs_outs = [
                dram.tile([n * 32, D], BF, name=f"rs_out{c}", tag=f"rs_out{c}")
                for c, n in enumerate(RS_CHUNKS)
            ]

            ident_bf = constp.tile([128, 128], BF, tag="ident_bf")
            make_identity(nc, ident_bf[:])
            # transposed causal mask for S^T tiles: entry (k, q): keep when
            # q >= k, else -1e10
            cmaskT = constp.tile([128, 128], FP, tag="cmaskT")
            nc.gpsimd.memset(cmaskT[:], 0.0)
            nc.gpsimd.affine_select(
                out=cmaskT[:],
                in_=cmaskT[:],
                compare_op=mybir.AluOpType.is_ge,
                fill=-1e10,
                base=0,
                # keep when (-k + q) >= 0
                pattern=[[1, 128]],
                channel_multiplier=-1,
            )
            # Q^T/K^T: [128 (head dim, even|odd basis), HPC*S], block (h, st)
            # at free offset h*S + st*128.
            QT = resp.tile([128, HPC * S], BF, tag="QT")
            KT = resp.tile([128, HPC * S], BF, tag="KT")
            # V with appended ones column per (h, kt) block: [128, 129] blocks
            Vones = resp.tile([128, HPC * NQT * 129], BF, tag="Vones")
            nc.vector.memset(
                Vones[:].rearrange("p (b o) -> p b o", o=129)[:, :, 128:129], 1.0
            )

            # ---------------- fused QKV projection ----------------
            with (
                tc.tile_pool(name="wpool", bufs=1) as wpool,
                tc.tile_pool(name="xtp", bufs=3) as xtp,
                tc.tile_pool(name="ropep", bufs=3) as ropep,
                tc.tile_pool(name="qps", bufs=2, space="PSUM") as qps,
                tc.tile_pool(name="trps", bufs=2, space="PSUM") as trps,
            ):
                # cos/sin live in the phase-1 pool (freed before phase 2)
                # and ride the scalar HWDGE queue so they don't delay the
                # first weight/x chunks on the sync queue
                cos_sb = wpool.tile([128, NQT * 64], FP, tag="cos")
                sin_sb = wpool.tile([128, NQT * 64], FP, tag="sin")
                nc.scalar.dma_start(cos_sb[:], cosp[:])
                nc.scalar.dma_start(sin_sb[:], sinp[:])
                wq_sb = wpool.tile([128, NDC * HD], BF, tag="wq")
                wk_sb = wpool.tile([128, NDC * HD], BF, tag="wk")
                wv_sb = wpool.tile([128, NDC * HD], BF, tag="wv")
                # chunked weight loads, first chunks first so matmuls can
                # start as soon as xt[0] + the dc=0..3 weight slices land
                WCH = 4 * HD
                for wsb, wdr in ((wq_sb, wq), (wk_sb, wk), (wv_sb, wv)):
                    nc.sync.dma_start(wsb[:, 0:WCH], wdr[:, 0:WCH])
                xt_pre = {}
                for st in range(2):
                    xt_sb = xtp.tile([128, NDC * 128], BF, tag="xt")
                    nc.sync.dma_start(xt_sb[:], xT[st])
                    xt_pre[st] = xt_sb
                for c in range(1, 4):
                    for wsb, wdr in ((wq_sb, wq), (wk_sb, wk), (wv_sb, wv)):
                        nc.sync.dma_start(
                            wsb[:, c * WCH : (c + 1) * WCH],
                            wdr[:, c * WCH : (c + 1) * WCH],
                        )
                for st in range(NQT):
                    if st in xt_pre:
                        xt_sb = xt_pre[st]
                    else:
                        xt_sb = xtp.tile([128, NDC * 128], BF, tag="xt")
                        nc.sync.dma_start(xt_sb[:], xT[st])
                    ps_q = qps.tile([128, HD], FP, tag="psq")
                    ps_k = qps.tile([128, HD], FP, tag="psk")
                    ps_v = qps.tile([128, HD], FP, tag="psv")
                    for dc in range(NDC):
                        lhs = xt_sb[:, dc * 128 : (dc + 1) * 128]
                        for w_sb, ps in ((wq_sb, ps_q), (wk_sb, ps_k), (wv_sb, ps_v)):
                            nc.tensor.matmul(
                                ps[:, :],
                                lhs,
                                w_sb[:, dc * HD : (dc + 1) * HD],
                                start=(dc == 0),
                                stop=(dc == NDC - 1),
                            )
                    # V: copy per-head blocks into Vones (cast to bf16)
                    for h in range(HPC):
                        nc.vector.tensor_copy(
                            Vones[:, (h * NQT + st) * 129 : (h * NQT + st) * 129 + 128],
                            ps_v[:, h * 128 : (h + 1) * 128],
                        )
                    # RoPE on Q and K (head dims pre-permuted to even|odd
                    # halves via host-side W column permutation)
                    for ps, dst in ((ps_q, QT), (ps_k, KT)):
                        rot = ropep.tile([128, HD], BF, tag="rot")
                        tmp = ropep.tile([128, HD], FP, tag="tmp")
                        cc = (
                            cos_sb[:, st * 64 : (st + 1) * 64]
                            .rearrange("p (o f) -> p o f", o=1)
                            .broadcast_to((128, HPC, 64))
                        )
                        ss = (
                            sin_sb[:, st * 64 : (st + 1) * 64]
                            .rearrange("p (o f) -> p o f", o=1)
                            .broadcast_to((128, HPC, 64))
                        )
                        psv = ps[:].rearrange("p (h f) -> p h f", h=HPC)
                        rotv = rot[:].rearrange("p (h f) -> p h f", h=HPC)
                        tmpv = tmp[:].rearrange("p (h f) -> p h f", h=HPC)
                        x1 = psv[:, :, 0:64]
                        x2 = psv[:, :, 64:128]
                        t1 = tmpv[:, :, 0:64]
                        t2 = tmpv[:, :, 64:128]
                        nc.vector.tensor_mul(t1, x1, cc)
                        nc.vector.tensor_mul(t2, x2, ss)
                        nc.vector.tensor_sub(rotv[:, :, 0:64], t1, t2)
                        nc.vector.tensor_mul(t1, x1, ss)
                        nc.vector.tensor_mul(t2, x2, cc)
                        nc.vector.tensor_add(rotv[:, :, 64:128], t1, t2)
                        for h in range(HPC):
                            pt = trps.tile([128, 128], BF, tag="tr")
                            nc.tensor.transpose(
                                pt[:], rot[:, h * 128 : (h + 1) * 128], ident_bf[:]
                            )
                            nc.vector.tensor_copy(
                                dst[:, h * S + st * 128 : h * S + (st + 1) * 128],
                                pt[:],
                            )

            # ---------------- attention + output projection ----------------
            with (
                tc.tile_pool(name="wop", bufs=1) as wop,
                tc.tile_pool(name="atp", bufs=8) as atp,
                tc.tile_pool(name="outp", bufs=4) as outp,
                tc.tile_pool(name="statp", bufs=4) as statp,
                tc.tile_pool(name="onp", bufs=4) as onp,
                tc.tile_pool(name="atallp", bufs=2) as atallp,
                tc.tile_pool(name="sps", bufs=3, space="PSUM") as sps,
                tc.tile_pool(name="orp", bufs=2, space="PSUM") as orp,
                tc.tile_pool(name="trp", bufs=1, space="PSUM") as trp,
                tc.tile_pool(name="ppp", bufs=2, space="PSUM") as ppp,
            ):
                wo_sb = wop.tile([128, HPC * D], BF, tag="wo")
                nc.sync.dma_start(wo_sb[:], wo[:])
                done_qts = set()
                for qb in BLOCK_ORDER:
                    # per-block A^T buffer from a 2-deep ring: consecutive
                    # blocks use different buffers, so block N+1's scores
                    # never wait on block N's PV reads
                    ATall = atallp.tile([128, HPC * NQT * 512], BF, tag="ATall")
                    q0 = qb * QB * 128  # block's first global q (elements)
                    # scores^T + exp -> A^T slots, per (h, kt)
                    for h in range(HPC):
                        for kt in range(QB * qb + QB):
                            qlo = max(kt * 128, q0)
                            n = q0 + QB * 128 - qlo
                            ps_s = sps.tile([128, 512], FP, tag="scores")
                            nc.tensor.matmul(
                                ps_s[:, :n],
                                KT[:, h * S + kt * 128 : h * S + (kt + 1) * 128],
                                QT[:, h * S + qlo : h * S + q0 + QB * 128],
                                start=True,
                                stop=True,
                            )
                            if kt * 128 >= q0:  # diagonal tile
                                nc.vector.tensor_add(
                                    ps_s[:, 0:128], ps_s[:, 0:128], cmaskT[:]
                                )
                            slot = (h * NQT + kt) * 512
                            off = qlo - q0
                            nc.scalar.activation(
                                ATall[:, slot + off : slot + 512],
                                ps_s[:, :n],
                                mybir.ActivationFunctionType.Exp,
                                bias=0.0,
                                scale=SCALE,
                            )
                    # PV + normalize + transpose + projection, per q-tile
                    for qt in range(QB * qb, QB * qb + QB):
                        at_tiles = []
                        for h in range(HPC):
                            ps_or = orp.tile([128, 129], FP, tag="pv")
                            for kt in range(qt + 1):
                                slot = (h * NQT + kt) * 512
                                col = (qt - QB * qb) * 128
                                nc.tensor.matmul(
                                    ps_or[:, :],
                                    ATall[:, slot + col : slot + col + 128],
                                    Vones[
                                        :,
                                        (h * NQT + kt) * 129 : (h * NQT + kt) * 129
                                        + 129,
                                    ],
                                    start=(kt == 0),
                                    stop=(kt == qt),
                                    skip_group_check=True,
                                )
                            rinv = statp.tile([128, 1], FP, tag="rinv")
                            nc.vector.reciprocal(rinv[:], ps_or[:, 128:129])
                            onrm = onp.tile([128, 128], BF, tag="onrm")
                            nc.vector.tensor_scalar_mul(
                                onrm[:], ps_or[:, 0:128], rinv[:]
                            )
                            ptt = trp.tile([128, 128], BF, tag="ot")
                            nc.tensor.transpose(ptt[:], onrm[:], ident_bf[:])
                            atile = atp.tile([128, 128], BF, tag="at")
                            nc.vector.tensor_copy(atile[:], ptt[:])
                            at_tiles.append(atile)
                        c = next(i for i, qs in enumerate(CHUNK_QTS) if qt in qs)
                        qoff = qt - min(CHUNK_QTS[c])
                        for nt in range(D // 512):
                            ps_p = ppp.tile([128, 512], FP, tag="proj")
                            for h in range(HPC):
                                nc.tensor.matmul(
                                    ps_p[:],
                                    at_tiles[h][:],
                                    wo_sb[:, h * D + nt * 512 : h * D + (nt + 1) * 512],
                                    start=(h == 0),
                                    stop=(h == HPC - 1),
                                )
                            osb = outp.tile([128, 512], BF, tag="osb")
                            nc.vector.tensor_copy(osb[:], ps_p[:])
                            nc.sync.dma_start(
                                partials[c][
                                    qoff * 128 : (qoff + 1) * 128,
                                    nt * 512 : (nt + 1) * 512,
                                ],
                                osb[:],
                            )
                        # fire each finished reduce-scatter chunk immediately;
                        # its out-DMA rides the gpsimd queue right behind it
                        # so no compute/DMA stream ever waits on a collective
                        done_qts.add(qt)
                        if all(q in done_qts for q in CHUNK_QTS[c]):
                            nc.gpsimd.collective_compute(
                                "ReduceScatter",
                                mybir.AluOpType.add,
                                replica_groups=[[0, 1, 2, 3], [4, 5, 6, 7]],
                                ins=[partials[c].opt()],
                                outs=[rs_outs[c].opt()],
                            )
                            ooff = min(CHUNK_QTS[c]) * 32
                            nc.gpsimd.dma_start(
                                out[ooff : ooff + RS_CHUNKS[c] * 32, :],
                                rs_outs[c][:, :],
                            )

    n = _legalize_waits(nc)
    print(f"kernel: split {n} excess sync waits", file=sys.stderr)
    return nc


_NC_CACHE = None
LAST_RESULTS = None


def _ensure_ntff_hook():
    """The agent image's antenv lacks ``axon_hooks``, so the boot-time NTFF
    profile hook registration silently degrades and ``trace=True`` crashes
    on import.  Recreate the module and register the ctypes hook."""
    try:
        from antenv.axon_hooks import get_axon_ntff_profile_hook  # noqa: F401

        return
    except ImportError:
        pass
    import types

    import antenv

    mod = types.ModuleType("antenv.axon_hooks")
    _hook = [None]
    mod.set_axon_ntff_profile_hook = lambda h: _hook.__setitem__(0, h)
    mod.get_axon_ntff_profile_hook = lambda: _hook[0]
    sys.modules["antenv.axon_hooks"] = mod
    antenv.axon_hooks = mod
    if "/root/.axon_site" not in sys.path:
        sys.path.insert(0, "/root/.axon_site")
    from trn_agent_boot.trn_boot import _ntff_profile_via_ctypes

    mod.set_axon_ntff_profile_hook(
        _ntff_profile_via_ctypes("/opt/axon/libaxon_pjrt.so")
    )


def _get_nc():
    global _NC_CACHE
    if _NC_CACHE is None:
        _NC_CACHE = build_nc()
    return _NC_CACHE


def _shard_inputs(x, Wq, Wk, Wv, Wo, token_position):
    import ml_dtypes

    bf16 = ml_dtypes.bfloat16
    x = np.asarray(x, dtype=np.float32)
    Wq = np.asarray(Wq, dtype=np.float32)
    Wk = np.asarray(Wk, dtype=np.float32)
    Wv = np.asarray(Wv, dtype=np.float32)
    Wo = np.asarray(Wo, dtype=np.float32)
    pos = np.asarray(token_position)

    inv_freq = (1.0 / (THETA ** (np.arange(0, DKV, 2, dtype=np.float32) / DKV))).astype(
        np.float32
    )
    ang = pos.astype(np.float32)[:, None] * inv_freq[None, :]
    # host layout [128, NQT*64]: partition p, block st
    cos = np.cos(ang).astype(np.float32).reshape(NQT, 128, 64).transpose(1, 0, 2)
    sin = np.sin(ang).astype(np.float32).reshape(NQT, 128, 64).transpose(1, 0, 2)
    cos = np.ascontiguousarray(cos.reshape(128, NQT * 64))
    sin = np.ascontiguousarray(sin.reshape(128, NQT * 64))

    # per-head even|odd column permutation for RoPE half-split basis
    perm1 = np.concatenate([np.arange(0, DKV, 2), np.arange(1, DKV, 2)])

    def wlayout(w):  # [D, HD] -> [128, NDC*HD]
        return np.ascontiguousarray(
            w.reshape(NDC, 128, HD).transpose(1, 0, 2).reshape(128, NDC * HD)
        ).astype(bf16)

    in_maps = []
    xT_cache = {}
    for c in range(N_CORES):
        b, g = divmod(c, TP)
        hs = slice(g * HD, (g + 1) * HD)
        permg = np.concatenate([h * DKV + perm1 for h in range(HPC)])
        if b not in xT_cache:
            # [NQT, 128, NDC*128]: element (st, p, c*128+s) = x[b][st*128+s, c*128+p]
            xT_cache[b] = np.ascontiguousarray(
                x[b]
                .T.reshape(NDC, 128, NQT, 128)
                .transpose(2, 1, 0, 3)
                .reshape(NQT, 128, NDC * 128)
            ).astype(bf16)
        wo_g = np.ascontiguousarray(
            Wo[hs, :].reshape(HPC, 128, D).transpose(1, 0, 2).reshape(128, HPC * D)
        ).astype(bf16)
        in_maps.append(
            {
                "xT": xT_cache[b],
                "wq": wlayout(Wq[:, hs][:, permg]),
                "wk": wlayout(Wk[:, hs][:, permg]),
                "wv": wlayout(Wv[:, hs]),
                "wo": wo_g,
                "cosp": cos,
                "sinp": sin,
            }
        )
    return in_maps


def kernel(x, Wq, Wk, Wv, Wo, token_position, trace=False, trace_cores=None):
    global LAST_RESULTS
    if trace:
        _ensure_ntff_hook()
    nc = _get_nc()
    in_maps = _shard_inputs(x, Wq, Wk, Wv, Wo, token_position)
    res = run_bass_kernel_spmd(
        nc,
        in_maps,
        core_ids=list(range(N_CORES)),
        trace=trace,
        trace_cores=trace_cores,
    )
    LAST_RESULTS = res
    out = np.empty((B, S, D), dtype=np.float32)
    for core in range(N_CORES):
        b, g = divmod(core, TP)
        shard = np.asarray(res.results[core]["out"], dtype=np.float32)  # [S//TP, D]
        for qs in CHUNK_QTS:
            lo = min(qs)
            rows = len(qs) * 32
            gstart = lo * 128 + g * rows
            out[b, gstart : gstart + rows, :] = shard[lo * 32 : lo * 32 + rows, :]
    return out


# revision 30
# speedup vs baseline: 1.1191x; 1.1191x over previous
"""Distributed causal RoPE attention for Trainium2 (8 NeuronCores).

Mesh: 2 (batch) x 4 (head-group tensor-parallel).
Core c = b*4 + g handles batch b, heads [4g, 4g+4).

v2 design (vs. v1 baseline):
  - bf16 operands on the PE everywhere (PSUM accumulation stays fp32);
    fp32 error headroom is ~50x the tolerance, bf16 lands well inside it.
  - Fused single-pass QKV: one xT stream, each x tile stationary for
    three matmuls (Q/K/V) -> half the HBM traffic of the two-pass v1.
  - Scores computed TRANSPOSED: S^T[k, q] = (K tile).T stationary @ Q^T
    streaming, exp on ScalarE writes A^T tiles directly -> the 544 PE
    transposes + DVE casts of attn tiles in v1 are gone entirely.
  - PV: lhsT = A^T tile (stationary), rhs = [V | ones] so the PSUM
    output is [O | rowsum] -- softmax denominator comes out of the
    matmul for free as column 128, normalized with a cheap per-partition
    scalar multiply.
  - Projection partials + ReduceScatter in bf16 (CCE supports bf16):
    halves the collective time; host casts the bf16 output to fp32.
  - Host pre-lays-out every input so each tensor is ONE contiguous DMA.
"""

import sys

sys.path.insert(0, "/opt/trn_rl_repo")

import numpy as np

import concourse.bass as bass
import concourse.mybir as mybir
import concourse.tile as tile
from concourse.bass_utils import run_bass_kernel_spmd
from concourse.tile import add_dep_helper
from concourse.masks import make_identity

FP = mybir.dt.float32
BF = mybir.dt.bfloat16
D = 2048  # d_model
S = 2048  # sequence length
B = 2  # batch
NH = 16  # heads
DKV = 128  # head dim
THETA = 10000.0
TP = 4  # head-parallel groups
HPC = NH // TP  # heads per core = 4
HD = HPC * DKV  # head dims per core = 512
NQT = S // 128  # 16 query tiles
NDC = D // 128  # 16 contraction chunks
QB = 4  # q-tiles per attention block
NB = NQT // QB  # 4 blocks
SCALE = 1.0 / float(np.sqrt(DKV))
N_CORES = 8

# attention blocks are computed in order 0,1,3,2 so the reduce-scatter
# pipeline drains early; each chunk is a contiguous run of q-tiles and
# fires as soon as its last q-tile (in compute order) is projected.  The
# final chunk is a single q-tile so the end-of-kernel collective is tiny.
BLOCK_ORDER = [0, 1, 2, 3]
CHUNK_QTS = [[0, 1, 2], [3, 4, 5, 6], [7, 8, 9, 10], [11, 12, 13], [14, 15]]
RS_CHUNKS = [len(q) for q in CHUNK_QTS]


def _legalize_waits(nc):
    """This walrus build only accepts one embedded sync-wait per TPB
    instruction ("Too many sync wait commands").  Split excess waits of
    compute-engine instructions into preceding engine-local NoOps, each
    carrying a single wait.  DMA (queue-embedded) waits are left alone.
    """
    n_split = 0
    for f in nc.m.functions:
        for bb in f.blocks:
            out = []
            for ins in bb.instructions:
                si = ins.sync_info
                if (
                    si is not None
                    and len(si.on_wait) > 1
                    and ins.engine != mybir.EngineType.Unassigned
                ):
                    waits = {}
                    for w in si.on_wait:
                        key = (w.sync_type, w.id, w.wait_mode)
                        if key not in waits or (
                            w.wait_value is not None
                            and waits[key].wait_value is not None
                            and w.wait_value > waits[key].wait_value
                        ):
                            waits[key] = w
                    waits = list(waits.values())
                    for w in waits[:-1]:
                        nop = mybir.InstNoOp(name=f"{ins.name}-waitsplit-{n_split}")
                        n_split += 1
                        nop.engine = ins.engine
                        nop.sync_info = mybir.SyncInfo(on_wait=[w], on_update=[])
                        out.append(nop)
                    ins.sync_info = mybir.SyncInfo(
                        on_wait=[waits[-1]], on_update=si.on_update
                    )
                out.append(ins)
            bb.instructions = out
    return n_split


def build_nc():
    nc = bass.Bass()

    # Host-pre-transposed, bf16, each a single contiguous DMA.
    xT = nc.declare_dram_parameter("xT", [NQT, 128, NDC * 128], BF, isOutput=False)
    wq = nc.declare_dram_parameter("wq", [128, NDC * HD], BF, isOutput=False)
    wk = nc.declare_dram_parameter("wk", [128, NDC * HD], BF, isOutput=False)
    wv = nc.declare_dram_parameter("wv", [128, NDC * HD], BF, isOutput=False)
    wo = nc.declare_dram_parameter("wo", [128, HPC * D], BF, isOutput=False)
    cosp = nc.declare_dram_parameter("cosp", [128, NQT * 64], FP, isOutput=False)
    sinp = nc.declare_dram_parameter("sinp", [128, NQT * 64], FP, isOutput=False)
    out = nc.declare_dram_parameter("out", [S // TP, D], BF, isOutput=True)

    with tile.TileContext(nc) as tc:
        with (
            tc.tile_pool(name="dram", bufs=1, space="DRAM") as dram,
            tc.tile_pool(name="const", bufs=1) as constp,
            tc.tile_pool(name="resident", bufs=1) as resp,
        ):
            partials = [
                dram.tile([n * 128, D], BF, name=f"partial{c}", tag=f"partial{c}")
                for c, n in enumerate(RS_CHUNKS)
            ]
            rs_outs = [
                dram.tile([n * 32, D], BF, name=f"rs_out{c}", tag=f"rs_out{c}")
                for c, n in enumerate(RS_CHUNKS)
            ]

            ident_bf = constp.tile([128, 128], BF, tag="ident_bf")
            make_identity(nc, ident_bf[:])
            # transposed causal mask for S^T tiles: entry (k, q): keep when
            # q >= k, else -1e10
            cmaskT = constp.tile([128, 128], FP, tag="cmaskT")
            nc.gpsimd.memset(cmaskT[:], 0.0)
            nc.gpsimd.affine_select(
                out=cmaskT[:],
                in_=cmaskT[:],
                compare_op=mybir.AluOpType.is_ge,
                fill=-1e10,
                base=0,
                # keep when (-k + q) >= 0
                pattern=[[1, 128]],
                channel_multiplier=-1,
            )
            # Q^T/K^T: [128 (head dim, even|odd basis), HPC*S], block (h, st)
            # at free offset h*S + st*128.
            QT = resp.tile([128, HPC * S], BF, tag="QT")
            KT = resp.tile([128, HPC * S], BF, tag="KT")
            # V with appended ones column per (h, kt) block: [128, 129] blocks
            Vones = resp.tile([128, HPC * NQT * 129], BF, tag="Vones")
            nc.vector.memset(
                Vones[:].rearrange("p (b o) -> p b o", o=129)[:, :, 128:129], 1.0
            )

            # ---------------- fused QKV projection ----------------
            with (
                tc.tile_pool(name="wpool", bufs=1) as wpool,
                tc.tile_pool(name="xtp", bufs=3) as xtp,
                tc.tile_pool(name="ropep", bufs=3) as ropep,
                tc.tile_pool(name="qps", bufs=2, space="PSUM") as qps,
                tc.tile_pool(name="trps", bufs=2, space="PSUM") as trps,
            ):
                # cos/sin live in the phase-1 pool (freed before phase 2)
                # and ride the scalar HWDGE queue so they don't delay the
                # first weight/x chunks on the sync queue
                cos_sb = wpool.tile([128, NQT * 64], FP, tag="cos")
                sin_sb = wpool.tile([128, NQT * 64], FP, tag="sin")
                nc.scalar.dma_start(cos_sb[:], cosp[:])
                nc.scalar.dma_start(sin_sb[:], sinp[:])
                wq_sb = wpool.tile([128, NDC * HD], BF, tag="wq")
                wk_sb = wpool.tile([128, NDC * HD], BF, tag="wk")
                wv_sb = wpool.tile([128, NDC * HD], BF, tag="wv")
                # chunked weight loads, first chunks first so matmuls can
                # start as soon as xt[0] + the dc=0..3 weight slices land
                WCH = 4 * HD
                for wsb, wdr in ((wq_sb, wq), (wk_sb, wk), (wv_sb, wv)):
                    nc.sync.dma_start(wsb[:, 0:WCH], wdr[:, 0:WCH])
                xt_pre = {}
                for st in range(2):
                    xt_sb = xtp.tile([128, NDC * 128], BF, tag="xt")
                    nc.sync.dma_start(xt_sb[:], xT[st])
                    xt_pre[st] = xt_sb
                for c in range(1, 4):
                    for wsb, wdr in ((wq_sb, wq), (wk_sb, wk), (wv_sb, wv)):
                        nc.sync.dma_start(
                            wsb[:, c * WCH : (c + 1) * WCH],
                            wdr[:, c * WCH : (c + 1) * WCH],
                        )
                for st in range(NQT):
                    if st in xt_pre:
                        xt_sb = xt_pre[st]
                    else:
                        xt_sb = xtp.tile([128, NDC * 128], BF, tag="xt")
                        nc.sync.dma_start(xt_sb[:], xT[st])
                    ps_q = qps.tile([128, HD], FP, tag="psq")
                    ps_k = qps.tile([128, HD], FP, tag="psk")
                    ps_v = qps.tile([128, HD], FP, tag="psv")
                    for dc in range(NDC):
                        lhs = xt_sb[:, dc * 128 : (dc + 1) * 128]
                        for w_sb, ps in ((wq_sb, ps_q), (wk_sb, ps_k), (wv_sb, ps_v)):
                            nc.tensor.matmul(
                                ps[:, :],
                                lhs,
                                w_sb[:, dc * HD : (dc + 1) * HD],
                                start=(dc == 0),
                                stop=(dc == NDC - 1),
                            )
                    # V: copy per-head blocks into Vones (cast to bf16)
                    for h in range(HPC):
                        nc.vector.tensor_copy(
                            Vones[:, (h * NQT + st) * 129 : (h * NQT + st) * 129 + 128],
                            ps_v[:, h * 128 : (h + 1) * 128],
                        )
                    # RoPE on Q and K (head dims pre-permuted to even|odd
                    # halves via host-side W column permutation)
                    for ps, dst in ((ps_q, QT), (ps_k, KT)):
                        rot = ropep.tile([128, HD], BF, tag="rot")
                        tmp = ropep.tile([128, HD], FP, tag="tmp")
                        cc = (
                            cos_sb[:, st * 64 : (st + 1) * 64]
                            .rearrange("p (o f) -> p o f", o=1)
                            .broadcast_to((128, HPC, 64))
                        )
                        ss = (
                            sin_sb[:, st * 64 : (st + 1) * 64]
                            .rearrange("p (o f) -> p o f", o=1)
                            .broadcast_to((128, HPC, 64))
                        )
                        psv = ps[:].rearrange("p (h f) -> p h f", h=HPC)
                        rotv = rot[:].rearrange("p (h f) -> p h f", h=HPC)
                        tmpv = tmp[:].rearrange("p (h f) -> p h f", h=HPC)
                        x1 = psv[:, :, 0:64]
                        x2 = psv[:, :, 64:128]
                        t1 = tmpv[:, :, 0:64]
                        t2 = tmpv[:, :, 64:128]
                        nc.vector.tensor_mul(t1, x1, cc)
                        nc.vector.tensor_mul(t2, x2, ss)
                        nc.vector.tensor_sub(rotv[:, :, 0:64], t1, t2)
                        nc.vector.tensor_mul(t1, x1, ss)
                        nc.vector.tensor_mul(t2, x2, cc)
                        nc.vector.tensor_add(rotv[:, :, 64:128], t1, t2)
                        for h in range(HPC):
                            pt = trps.tile([128, 128], BF, tag="tr")
                            nc.tensor.transpose(
                                pt[:], rot[:, h * 128 : (h + 1) * 128], ident_bf[:]
                            )
                            nc.vector.tensor_copy(
                                dst[:, h * S + st * 128 : h * S + (st + 1) * 128],
                                pt[:],
                            )

            # ---------------- attention + output projection ----------------
            with (
                tc.tile_pool(name="wop", bufs=1) as wop,
                tc.tile_pool(name="atp", bufs=8) as atp,
                tc.tile_pool(name="outp", bufs=4) as outp,
                tc.tile_pool(name="statp", bufs=4) as statp,
                tc.tile_pool(name="onp", bufs=4) as onp,
                tc.tile_pool(name="atallp", bufs=2) as atallp,
                tc.tile_pool(name="sps", bufs=3, space="PSUM") as sps,
                tc.tile_pool(name="orp", bufs=2, space="PSUM") as orp,
                tc.tile_pool(name="trp", bufs=1, space="PSUM") as trp,
                tc.tile_pool(name="ppp", bufs=2, space="PSUM") as ppp,
            ):
                wo_sb = wop.tile([128, HPC * D], BF, tag="wo")
                nc.sync.dma_start(wo_sb[:], wo[:])
                done_qts = set()
                for qb in BLOCK_ORDER:
                    # per-block A^T buffer from a 2-deep ring: consecutive
                    # blocks use different buffers, so block N+1's scores
                    # never wait on block N's PV reads
                    ATall = atallp.tile([128, HPC * NQT * 512], BF, tag="ATall")
                    q0 = qb * QB * 128  # block's first global q (elements)
                    # scores^T + exp -> A^T slots, per (h, kt)
                    for h in range(HPC):
                        for kt in range(QB * qb + QB):
                            qlo = max(kt * 128, q0)
                            n = q0 + QB * 128 - qlo
                            ps_s = sps.tile([128, 512], FP, tag="scores")
                            nc.tensor.matmul(
                                ps_s[:, :n],
                                KT[:, h * S + kt * 128 : h * S + (kt + 1) * 128],
                                QT[:, h * S + qlo : h * S + q0 + QB * 128],
                                start=True,
                                stop=True,
                            )
                            if kt * 128 >= q0:  # diagonal tile
                                nc.vector.tensor_add(
                                    ps_s[:, 0:128], ps_s[:, 0:128], cmaskT[:]
                                )
                            slot = (h * NQT + kt) * 512
                            off = qlo - q0
                            nc.scalar.activation(
                                ATall[:, slot + off : slot + 512],
                                ps_s[:, :n],
                                mybir.ActivationFunctionType.Exp,
                                bias=0.0,
                                scale=SCALE,
                            )
                    # PV + normalize + transpose + projection, per q-tile
                    for qt in range(QB * qb, QB * qb + QB):
                        at_tiles = []
                        for h in range(HPC):
                            ps_or = orp.tile([128, 129], FP, tag="pv")
                            for kt in range(qt + 1):
                                slot = (h * NQT + kt) * 512
                                col = (qt - QB * qb) * 128
                                nc.tensor.matmul(
                                    ps_or[:, :],
                                    ATall[:, slot + col : slot + col + 128],
                                    Vones[
                                        :,
                                        (h * NQT + kt) * 129 : (h * NQT + kt) * 129
                                        + 129,
                                    ],
                                    start=(kt == 0),
                                    stop=(kt == qt),
                                    skip_group_check=True,
                                )
                            rinv = statp.tile([128, 1], FP, tag="rinv")
                            nc.vector.reciprocal(rinv[:], ps_or[:, 128:129])
                            onrm = onp.tile([128, 128], BF, tag="onrm")
                            nc.vector.tensor_scalar_mul(
                                onrm[:], ps_or[:, 0:128], rinv[:]
                            )
                            ptt = trp.tile([128, 128], BF, tag="ot")
                            nc.tensor.transpose(ptt[:], onrm[:], ident_bf[:])
                            atile = atp.tile([128, 128], BF, tag="at")
                            nc.vector.tensor_copy(atile[:], ptt[:])
                            at_tiles.append(atile)
                        c = next(i for i, qs in enumerate(CHUNK_QTS) if qt in qs)
                        qoff = qt - min(CHUNK_QTS[c])
                        for nt in range(D // 512):
                            ps_p = ppp.tile([128, 512], FP, tag="proj")
                            for h in range(HPC):
                                nc.tensor.matmul(
                                    ps_p[:],
                                    at_tiles[h][:],
                                    wo_sb[:, h * D + nt * 512 : h * D + (nt + 1) * 512],
                                    start=(h == 0),
                                    stop=(h == HPC - 1),
                                )
                            osb = outp.tile([128, 512], BF, tag="osb")
                            nc.vector.tensor_copy(osb[:], ps_p[:])
                            nc.sync.dma_start(
                                partials[c][
                                    qoff * 128 : (qoff + 1) * 128,
                                    nt * 512 : (nt + 1) * 512,
                                ],
                                osb[:],
                            )
                        # fire each finished reduce-scatter chunk immediately;
                        # its out-DMA rides the gpsimd queue right behind it
                        # so no compute/DMA stream ever waits on a collective
                        done_qts.add(qt)
                        if all(q in done_qts for q in CHUNK_QTS[c]):
                            nc.gpsimd.collective_compute(
                                "ReduceScatter",
                                mybir.AluOpType.add,
                                replica_groups=[[0, 1, 2, 3], [4, 5, 6, 7]],
                                ins=[partials[c].opt()],
                                outs=[rs_outs[c].opt()],
                            )
                            ooff = min(CHUNK_QTS[c]) * 32
                            nc.gpsimd.dma_start(
                                out[ooff : ooff + RS_CHUNKS[c] * 32, :],
                                rs_outs[c][:, :],
                            )

    n = _legalize_waits(nc)
    print(f"kernel: split {n} excess sync waits", file=sys.stderr)
    return nc


_NC_CACHE = None
LAST_RESULTS = None


def _ensure_ntff_hook():
    """The agent image's antenv lacks ``axon_hooks``, so the boot-time NTFF
    profile hook registration silently degrades and ``trace=True`` crashes
    on import.  Recreate the module and register the ctypes hook."""
    try:
        from antenv.axon_hooks import get_axon_ntff_profile_hook  # noqa: F401

        return
    except ImportError:
        pass
    import types

    import antenv

    mod = types.ModuleType("antenv.axon_hooks")
    _hook = [None]
    mod.set_axon_ntff_profile_hook = lambda h: _hook.__setitem__(0, h)
    mod.get_axon_ntff_profile_hook = lambda: _hook[0]
    sys.modules["antenv.axon_hooks"] = mod
    antenv.axon_hooks = mod
    if "/root/.axon_site" not in sys.path:
        sys.path.insert(0, "/root/.axon_site")
    from trn_agent_boot.trn_boot import _ntff_profile_via_ctypes

    mod.set_axon_ntff_profile_hook(
        _ntff_profile_via_ctypes("/opt/axon/libaxon_pjrt.so")
    )


def _get_nc():
    global _NC_CACHE
    if _NC_CACHE is None:
        _NC_CACHE = build_nc()
    return _NC_CACHE


def _shard_inputs(x, Wq, Wk, Wv, Wo, token_position):
    import ml_dtypes

    bf16 = ml_dtypes.bfloat16
    x = np.asarray(x, dtype=np.float32)
    Wq = np.asarray(Wq, dtype=np.float32)
    Wk = np.asarray(Wk, dtype=np.float32)
    Wv = np.asarray(Wv, dtype=np.float32)
    Wo = np.asarray(Wo, dtype=np.float32)
    pos = np.asarray(token_position)

    inv_freq = (1.0 / (THETA ** (np.arange(0, DKV, 2, dtype=np.float32) / DKV))).astype(
        np.float32
    )
    ang = pos.astype(np.float32)[:, None] * inv_freq[None, :]
    # host layout [128, NQT*64]: partition p, block st
    cos = np.cos(ang).astype(np.float32).reshape(NQT, 128, 64).transpose(1, 0, 2)
    sin = np.sin(ang).astype(np.float32).reshape(NQT, 128, 64).transpose(1, 0, 2)
    cos = np.ascontiguousarray(cos.reshape(128, NQT * 64))
    sin = np.ascontiguousarray(sin.reshape(128, NQT * 64))

    # per-head even|odd column permutation for RoPE half-split basis
    perm1 = np.concatenate([np.arange(0, DKV, 2), np.arange(1, DKV, 2)])

    def wlayout(w):  # [D, HD] -> [128, NDC*HD]
        return np.ascontiguousarray(
            w.reshape(NDC, 128, HD).transpose(1, 0, 2).reshape(128, NDC * HD)
        ).astype(bf16)

    in_maps = []
    xT_cache = {}
    for c in range(N_CORES):
        b, g = divmod(c, TP)
        hs = slice(g * HD, (g + 1) * HD)
        permg = np.concatenate([h * DKV + perm1 for h in range(HPC)])
        if b not in xT_cache:
            # [NQT, 128, NDC*128]: element (st, p, c*128+s) = x[b][st*128+s, c*128+p]
            xT_cache[b] = np.ascontiguousarray(
                x[b]
                .T.reshape(NDC, 128, NQT, 128)
                .transpose(2, 1, 0, 3)
                .reshape(NQT, 128, NDC * 128)
            ).astype(bf16)
        wo_g = np.ascontiguousarray(
            Wo[hs, :].reshape(HPC, 128, D).transpose(1, 0, 2).reshape(128, HPC * D)
        ).astype(bf16)
        in_maps.append(
            {
                "xT": xT_cache[b],
                "wq": wlayout(Wq[:, hs][:, permg]),
                "wk": wlayout(Wk[:, hs][:, permg]),
                "wv": wlayout(Wv[:, hs]),
                "wo": wo_g,
                "cosp": cos,
                "sinp": sin,
            }
        )
    return in_maps


def kernel(x, Wq, Wk, Wv, Wo, token_position, trace=False, trace_cores=None):
    global LAST_RESULTS
    if trace:
        _ensure_ntff_hook()
    nc = _get_nc()
    in_maps = _shard_inputs(x, Wq, Wk, Wv, Wo, token_position)
    res = run_bass_kernel_spmd(
        nc,
        in_maps,
        core_ids=list(range(N_CORES)),
        trace=trace,
        trace_cores=trace_cores,
    )
    LAST_RESULTS = res
    out = np.empty((B, S, D), dtype=np.float32)
    for core in range(N_CORES):
        b, g = divmod(core, TP)
        shard = np.asarray(res.results[core]["out"], dtype=np.float32)  # [S//TP, D]
        for qs in CHUNK_QTS:
            lo = min(qs)
            rows = len(qs) * 32
            gstart = lo * 128 + g * rows
            out[b, gstart : gstart + rows, :] = shard[lo * 32 : lo * 32 + rows, :]
    return out


# revision 34
# speedup vs baseline: 1.1288x; 1.0087x over previous
"""Distributed causal RoPE attention for Trainium2 (8 NeuronCores).

Mesh: 2 (batch) x 4 (head-group tensor-parallel).
Core c = b*4 + g handles batch b, heads [4g, 4g+4).

v2 design (vs. v1 baseline):
  - bf16 operands on the PE everywhere (PSUM accumulation stays fp32);
    fp32 error headroom is ~50x the tolerance, bf16 lands well inside it.
  - Fused single-pass QKV: one xT stream, each x tile stationary for
    three matmuls (Q/K/V) -> half the HBM traffic of the two-pass v1.
  - Scores computed TRANSPOSED: S^T[k, q] = (K tile).T stationary @ Q^T
    streaming, exp on ScalarE writes A^T tiles directly -> the 544 PE
    transposes + DVE casts of attn tiles in v1 are gone entirely.
  - PV: lhsT = A^T tile (stationary), rhs = [V | ones] so the PSUM
    output is [O | rowsum] -- softmax denominator comes out of the
    matmul for free as column 128, normalized with a cheap per-partition
    scalar multiply.
  - Projection partials + ReduceScatter in bf16 (CCE supports bf16):
    halves the collective time; host casts the bf16 output to fp32.
  - Host pre-lays-out every input so each tensor is ONE contiguous DMA.
"""

import sys

sys.path.insert(0, "/opt/trn_rl_repo")

import numpy as np

import concourse.bass as bass
import concourse.mybir as mybir
import concourse.tile as tile
from concourse.bass_utils import run_bass_kernel_spmd
from concourse.tile import add_dep_helper
from concourse.masks import make_identity

FP = mybir.dt.float32
BF = mybir.dt.bfloat16
D = 2048  # d_model
S = 2048  # sequence length
B = 2  # batch
NH = 16  # heads
DKV = 128  # head dim
THETA = 10000.0
TP = 4  # head-parallel groups
HPC = NH // TP  # heads per core = 4
HD = HPC * DKV  # head dims per core = 512
NQT = S // 128  # 16 query tiles
NDC = D // 128  # 16 contraction chunks
QB = 4  # q-tiles per attention block
NB = NQT // QB  # 4 blocks
SCALE = 1.0 / float(np.sqrt(DKV))
N_CORES = 8

# attention blocks are computed in order 0,1,3,2 so the reduce-scatter
# pipeline drains early; each chunk is a contiguous run of q-tiles and
# fires as soon as its last q-tile (in compute order) is projected.  The
# final chunk is a single q-tile so the end-of-kernel collective is tiny.
BLOCK_ORDER = [0, 1, 2, 3]
CHUNK_QTS = [[0, 1, 2], [3, 4, 5, 6], [7, 8, 9, 10], [11, 12, 13], [14], [15]]
RS_CHUNKS = [len(q) for q in CHUNK_QTS]


def _legalize_waits(nc):
    """This walrus build only accepts one embedded sync-wait per TPB
    instruction ("Too many sync wait commands").  Split excess waits of
    compute-engine instructions into preceding engine-local NoOps, each
    carrying a single wait.  DMA (queue-embedded) waits are left alone.
    """
    n_split = 0
    for f in nc.m.functions:
        for bb in f.blocks:
            out = []
            for ins in bb.instructions:
                si = ins.sync_info
                if (
                    si is not None
                    and len(si.on_wait) > 1
                    and ins.engine != mybir.EngineType.Unassigned
                ):
                    waits = {}
                    for w in si.on_wait:
                        key = (w.sync_type, w.id, w.wait_mode)
                        if key not in waits or (
                            w.wait_value is not None
                            and waits[key].wait_value is not None
                            and w.wait_value > waits[key].wait_value
                        ):
                            waits[key] = w
                    waits = list(waits.values())
                    for w in waits[:-1]:
                        nop = mybir.InstNoOp(name=f"{ins.name}-waitsplit-{n_split}")
                        n_split += 1
                        nop.engine = ins.engine
                        nop.sync_info = mybir.SyncInfo(on_wait=[w], on_update=[])
                        out.append(nop)
                    ins.sync_info = mybir.SyncInfo(
                        on_wait=[waits[-1]], on_update=si.on_update
                    )
                out.append(ins)
            bb.instructions = out
    return n_split


def build_nc():
    nc = bass.Bass()

    # Host-pre-transposed, bf16, each a single contiguous DMA.
    xT = nc.declare_dram_parameter("xT", [NQT, 128, NDC * 128], BF, isOutput=False)
    wq = nc.declare_dram_parameter("wq", [128, NDC * HD], BF, isOutput=False)
    wk = nc.declare_dram_parameter("wk", [128, NDC * HD], BF, isOutput=False)
    wv = nc.declare_dram_parameter("wv", [128, NDC * HD], BF, isOutput=False)
    wo = nc.declare_dram_parameter("wo", [128, HPC * D], BF, isOutput=False)
    cosp = nc.declare_dram_parameter("cosp", [128, NQT * 64], FP, isOutput=False)
    sinp = nc.declare_dram_parameter("sinp", [128, NQT * 64], FP, isOutput=False)
    out = nc.declare_dram_parameter("out", [S // TP, D], BF, isOutput=True)

    with tile.TileContext(nc) as tc:
        with (
            tc.tile_pool(name="dram", bufs=1, space="DRAM") as dram,
            tc.tile_pool(name="const", bufs=1) as constp,
            tc.tile_pool(name="resident", bufs=1) as resp,
        ):
            partials = [
                dram.tile([n * 128, D], BF, name=f"partial{c}", tag=f"partial{c}")
                for c, n in enumerate(RS_CHUNKS)
            ]
            rs_outs = [
                dram.tile([n * 32, D], BF, name=f"rs_out{c}", tag=f"rs_out{c}")
                for c, n in enumerate(RS_CHUNKS)
            ]

            ident_bf = constp.tile([128, 128], BF, tag="ident_bf")
            make_identity(nc, ident_bf[:])
            # transposed causal mask for S^T tiles: entry (k, q): keep when
            # q >= k, else -1e10
            cmaskT = constp.tile([128, 128], FP, tag="cmaskT")
            nc.gpsimd.memset(cmaskT[:], 0.0)
            nc.gpsimd.affine_select(
                out=cmaskT[:],
                in_=cmaskT[:],
                compare_op=mybir.AluOpType.is_ge,
                fill=-1e10,
                base=0,
                # keep when (-k + q) >= 0
                pattern=[[1, 128]],
                channel_multiplier=-1,
            )
            # Q^T/K^T: [128 (head dim, even|odd basis), HPC*S], block (h, st)
            # at free offset h*S + st*128.
            QT = resp.tile([128, HPC * S], BF, tag="QT")
            KT = resp.tile([128, HPC * S], BF, tag="KT")
            # V with appended ones column per (h, kt) block: [128, 129] blocks
            Vones = resp.tile([128, HPC * NQT * 129], BF, tag="Vones")
            nc.vector.memset(
                Vones[:].rearrange("p (b o) -> p b o", o=129)[:, :, 128:129], 1.0
            )

            # ---------------- fused QKV projection ----------------
            with (
                tc.tile_pool(name="wpool", bufs=1) as wpool,
                tc.tile_pool(name="xtp", bufs=3) as xtp,
                tc.tile_pool(name="ropep", bufs=3) as ropep,
                tc.tile_pool(name="qps", bufs=2, space="PSUM") as qps,
                tc.tile_pool(name="trps", bufs=2, space="PSUM") as trps,
            ):
                # cos/sin live in the phase-1 pool (freed before phase 2)
                # and ride the scalar HWDGE queue so they don't delay the
                # first weight/x chunks on the sync queue
                cos_sb = wpool.tile([128, NQT * 64], FP, tag="cos")
                sin_sb = wpool.tile([128, NQT * 64], FP, tag="sin")
                nc.scalar.dma_start(cos_sb[:], cosp[:])
                nc.scalar.dma_start(sin_sb[:], sinp[:])
                wq_sb = wpool.tile([128, NDC * HD], BF, tag="wq")
                wk_sb = wpool.tile([128, NDC * HD], BF, tag="wk")
                wv_sb = wpool.tile([128, NDC * HD], BF, tag="wv")
                # chunked weight loads, first chunks first so matmuls can
                # start as soon as xt[0] + the dc=0..3 weight slices land
                WBND = [0, 2, 6, 11, 16]  # dc boundaries of the weight chunks
                for wsb, wdr in ((wq_sb, wq), (wk_sb, wk), (wv_sb, wv)):
                    nc.sync.dma_start(
                        wsb[:, 0 : WBND[1] * HD], wdr[:, 0 : WBND[1] * HD]
                    )
                xt_pre = {}
                for st in range(2):
                    xt_sb = xtp.tile([128, NDC * 128], BF, tag="xt")
                    nc.sync.dma_start(xt_sb[:], xT[st])
                    xt_pre[st] = xt_sb
                for c in range(1, 4):
                    for wsb, wdr in ((wq_sb, wq), (wk_sb, wk), (wv_sb, wv)):
                        nc.sync.dma_start(
                            wsb[:, WBND[c] * HD : WBND[c + 1] * HD],
                            wdr[:, WBND[c] * HD : WBND[c + 1] * HD],
                        )
                for st in range(NQT):
                    if st in xt_pre:
                        xt_sb = xt_pre[st]
                    else:
                        xt_sb = xtp.tile([128, NDC * 128], BF, tag="xt")
                        nc.sync.dma_start(xt_sb[:], xT[st])
                    ps_q = qps.tile([128, HD], FP, tag="psq")
                    ps_k = qps.tile([128, HD], FP, tag="psk")
                    ps_v = qps.tile([128, HD], FP, tag="psv")
                    for dc in range(NDC):
                        lhs = xt_sb[:, dc * 128 : (dc + 1) * 128]
                        for w_sb, ps in ((wq_sb, ps_q), (wk_sb, ps_k), (wv_sb, ps_v)):
                            nc.tensor.matmul(
                                ps[:, :],
                                lhs,
                                w_sb[:, dc * HD : (dc + 1) * HD],
                                start=(dc == 0),
                                stop=(dc == NDC - 1),
                            )
                    # V: copy per-head blocks into Vones (cast to bf16)
                    for h in range(HPC):
                        nc.vector.tensor_copy(
                            Vones[:, (h * NQT + st) * 129 : (h * NQT + st) * 129 + 128],
                            ps_v[:, h * 128 : (h + 1) * 128],
                        )
                    # RoPE on Q and K (head dims pre-permuted to even|odd
                    # halves via host-side W column permutation)
                    for ps, dst in ((ps_q, QT), (ps_k, KT)):
                        rot = ropep.tile([128, HD], BF, tag="rot")
                        tmp = ropep.tile([128, HD], FP, tag="tmp")
                        cc = (
                            cos_sb[:, st * 64 : (st + 1) * 64]
                            .rearrange("p (o f) -> p o f", o=1)
                            .broadcast_to((128, HPC, 64))
                        )
                        ss = (
                            sin_sb[:, st * 64 : (st + 1) * 64]
                            .rearrange("p (o f) -> p o f", o=1)
                            .broadcast_to((128, HPC, 64))
                        )
                        psv = ps[:].rearrange("p (h f) -> p h f", h=HPC)
                        rotv = rot[:].rearrange("p (h f) -> p h f", h=HPC)
                        tmpv = tmp[:].rearrange("p (h f) -> p h f", h=HPC)
                        x1 = psv[:, :, 0:64]
                        x2 = psv[:, :, 64:128]
                        t1 = tmpv[:, :, 0:64]
                        t2 = tmpv[:, :, 64:128]
                        nc.vector.tensor_mul(t1, x1, cc)
                        nc.vector.tensor_mul(t2, x2, ss)
                        nc.vector.tensor_sub(rotv[:, :, 0:64], t1, t2)
                        nc.vector.tensor_mul(t1, x1, ss)
                        nc.vector.tensor_mul(t2, x2, cc)
                        nc.vector.tensor_add(rotv[:, :, 64:128], t1, t2)
                        for h in range(HPC):
                            pt = trps.tile([128, 128], BF, tag="tr")
                            nc.tensor.transpose(
                                pt[:], rot[:, h * 128 : (h + 1) * 128], ident_bf[:]
                            )
                            nc.vector.tensor_copy(
                                dst[:, h * S + st * 128 : h * S + (st + 1) * 128],
                                pt[:],
                            )

            # ---------------- attention + output projection ----------------
            with (
                tc.tile_pool(name="wop", bufs=1) as wop,
                tc.tile_pool(name="atp", bufs=20) as atp,
                tc.tile_pool(name="outp", bufs=4) as outp,
                tc.tile_pool(name="statp", bufs=4) as statp,
                tc.tile_pool(name="onp", bufs=4) as onp,
                tc.tile_pool(name="atallp", bufs=2) as atallp,
                tc.tile_pool(name="sps", bufs=3, space="PSUM") as sps,
                tc.tile_pool(name="orp", bufs=2, space="PSUM") as orp,
                tc.tile_pool(name="trp", bufs=1, space="PSUM") as trp,
                tc.tile_pool(name="ppp", bufs=2, space="PSUM") as ppp,
            ):
                wo_sb = wop.tile([128, HPC * D], BF, tag="wo")
                nc.sync.dma_start(wo_sb[:], wo[:])
                done_qts = set()
                for qb in BLOCK_ORDER:
                    # per-block A^T buffer from a 2-deep ring: consecutive
                    # blocks use different buffers, so block N+1's scores
                    # never wait on block N's PV reads
                    ATall = atallp.tile([128, HPC * NQT * 512], BF, tag="ATall")
                    q0 = qb * QB * 128  # block's first global q (elements)
                    # scores^T + exp -> A^T slots, per (h, kt)
                    for h in range(HPC):
                        for kt in range(QB * qb + QB):
                            qlo = max(kt * 128, q0)
                            n = q0 + QB * 128 - qlo
                            ps_s = sps.tile([128, 512], FP, tag="scores")
                            nc.tensor.matmul(
                                ps_s[:, :n],
                                KT[:, h * S + kt * 128 : h * S + (kt + 1) * 128],
                                QT[:, h * S + qlo : h * S + q0 + QB * 128],
                                start=True,
                                stop=True,
                            )
                            if kt * 128 >= q0:  # diagonal tile
                                nc.vector.tensor_add(
                                    ps_s[:, 0:128], ps_s[:, 0:128], cmaskT[:]
                                )
                            slot = (h * NQT + kt) * 512
                            off = qlo - q0
                            nc.scalar.activation(
                                ATall[:, slot + off : slot + 512],
                                ps_s[:, :n],
                                mybir.ActivationFunctionType.Exp,
                                bias=0.0,
                                scale=SCALE,
                            )
                    # PV + normalize + transpose, h-OUTER: head h's PV only
                    # needs head h's exps, so the PE isn't stalled waiting
                    # for the whole ScalarE exp stream at block start
                    ats = {}
                    for h in range(HPC):
                        for qt in range(QB * qb, QB * qb + QB):
                            ps_or = orp.tile([128, 129], FP, tag="pv")
                            for kt in range(qt + 1):
                                slot = (h * NQT + kt) * 512
                                col = (qt - QB * qb) * 128
                                nc.tensor.matmul(
                                    ps_or[:, :],
                                    ATall[:, slot + col : slot + col + 128],
                                    Vones[
                                        :,
                                        (h * NQT + kt) * 129 : (h * NQT + kt) * 129
                                        + 129,
                                    ],
                                    start=(kt == 0),
                                    stop=(kt == qt),
                                    skip_group_check=True,
                                )
                            rinv = statp.tile([128, 1], FP, tag="rinv")
                            nc.vector.reciprocal(rinv[:], ps_or[:, 128:129])
                            onrm = onp.tile([128, 128], BF, tag="onrm")
                            nc.vector.tensor_scalar_mul(
                                onrm[:], ps_or[:, 0:128], rinv[:]
                            )
                            ptt = trp.tile([128, 128], BF, tag="ot")
                            nc.tensor.transpose(ptt[:], onrm[:], ident_bf[:])
                            atile = atp.tile([128, 128], BF, tag="at")
                            nc.vector.tensor_copy(atile[:], ptt[:])
                            ats[(qt, h)] = atile
                    # projection + partial DMAs, per q-tile
                    for qt in range(QB * qb, QB * qb + QB):
                        at_tiles = [ats[(qt, h)] for h in range(HPC)]
                        c = next(i for i, qs in enumerate(CHUNK_QTS) if qt in qs)
                        qoff = qt - min(CHUNK_QTS[c])
                        for nt in range(D // 512):
                            ps_p = ppp.tile([128, 512], FP, tag="proj")
                            for h in range(HPC):
                                nc.tensor.matmul(
                                    ps_p[:],
                                    at_tiles[h][:],
                                    wo_sb[:, h * D + nt * 512 : h * D + (nt + 1) * 512],
                                    start=(h == 0),
                                    stop=(h == HPC - 1),
                                )
                            osb = outp.tile([128, 512], BF, tag="osb")
                            nc.vector.tensor_copy(osb[:], ps_p[:])
                            nc.sync.dma_start(
                                partials[c][
                                    qoff * 128 : (qoff + 1) * 128,
                                    nt * 512 : (nt + 1) * 512,
                                ],
                                osb[:],
                            )
                        # fire each finished reduce-scatter chunk immediately;
                        # its out-DMA rides the gpsimd queue right behind it
                        # so no compute/DMA stream ever waits on a collective
                        done_qts.add(qt)
                        if all(q in done_qts for q in CHUNK_QTS[c]):
                            nc.gpsimd.collective_compute(
                                "ReduceScatter",
                                mybir.AluOpType.add,
                                replica_groups=[[0, 1, 2, 3], [4, 5, 6, 7]],
                                ins=[partials[c].opt()],
                                outs=[rs_outs[c].opt()],
                            )
                            ooff = min(CHUNK_QTS[c]) * 32
                            nc.gpsimd.dma_start(
                                out[ooff : ooff + RS_CHUNKS[c] * 32, :],
                                rs_outs[c][:, :],
                            )

    n = _legalize_waits(nc)
    print(f"kernel: split {n} excess sync waits", file=sys.stderr)
    return nc


_NC_CACHE = None
LAST_RESULTS = None


def _ensure_ntff_hook():
    """The agent image's antenv lacks ``axon_hooks``, so the boot-time NTFF
    profile hook registration silently degrades and ``trace=True`` crashes
    on import.  Recreate the module and register the ctypes hook."""
    try:
        from antenv.axon_hooks import get_axon_ntff_profile_hook  # noqa: F401

        return
    except ImportError:
        pass
    import types

    import antenv

    mod = types.ModuleType("antenv.axon_hooks")
    _hook = [None]
    mod.set_axon_ntff_profile_hook = lambda h: _hook.__setitem__(0, h)
    mod.get_axon_ntff_profile_hook = lambda: _hook[0]
    sys.modules["antenv.axon_hooks"] = mod
    antenv.axon_hooks = mod
    if "/root/.axon_site" not in sys.path:
        sys.path.insert(0, "/root/.axon_site")
    from trn_agent_boot.trn_boot import _ntff_profile_via_ctypes

    mod.set_axon_ntff_profile_hook(
        _ntff_profile_via_ctypes("/opt/axon/libaxon_pjrt.so")
    )


def _get_nc():
    global _NC_CACHE
    if _NC_CACHE is None:
        _NC_CACHE = build_nc()
    return _NC_CACHE


def _shard_inputs(x, Wq, Wk, Wv, Wo, token_position):
    import ml_dtypes

    bf16 = ml_dtypes.bfloat16
    x = np.asarray(x, dtype=np.float32)
    Wq = np.asarray(Wq, dtype=np.float32)
    Wk = np.asarray(Wk, dtype=np.float32)
    Wv = np.asarray(Wv, dtype=np.float32)
    Wo = np.asarray(Wo, dtype=np.float32)
    pos = np.asarray(token_position)

    inv_freq = (1.0 / (THETA ** (np.arange(0, DKV, 2, dtype=np.float32) / DKV))).astype(
        np.float32
    )
    ang = pos.astype(np.float32)[:, None] * inv_freq[None, :]
    # host layout [128, NQT*64]: partition p, block st
    cos = np.cos(ang).astype(np.float32).reshape(NQT, 128, 64).transpose(1, 0, 2)
    sin = np.sin(ang).astype(np.float32).reshape(NQT, 128, 64).transpose(1, 0, 2)
    cos = np.ascontiguousarray(cos.reshape(128, NQT * 64))
    sin = np.ascontiguousarray(sin.reshape(128, NQT * 64))

    # per-head even|odd column permutation for RoPE half-split basis
    perm1 = np.concatenate([np.arange(0, DKV, 2), np.arange(1, DKV, 2)])

    def wlayout(w):  # [D, HD] -> [128, NDC*HD]
        return np.ascontiguousarray(
            w.reshape(NDC, 128, HD).transpose(1, 0, 2).reshape(128, NDC * HD)
        ).astype(bf16)

    in_maps = []
    xT_cache = {}
    for c in range(N_CORES):
        b, g = divmod(c, TP)
        hs = slice(g * HD, (g + 1) * HD)
        permg = np.concatenate([h * DKV + perm1 for h in range(HPC)])
        if b not in xT_cache:
            # [NQT, 128, NDC*128]: element (st, p, c*128+s) = x[b][st*128+s, c*128+p]
            xT_cache[b] = np.ascontiguousarray(
                x[b]
                .T.reshape(NDC, 128, NQT, 128)
                .transpose(2, 1, 0, 3)
                .reshape(NQT, 128, NDC * 128)
            ).astype(bf16)
        wo_g = np.ascontiguousarray(
            Wo[hs, :].reshape(HPC, 128, D).transpose(1, 0, 2).reshape(128, HPC * D)
        ).astype(bf16)
        in_maps.append(
            {
                "xT": xT_cache[b],
                "wq": wlayout(Wq[:, hs][:, permg]),
                "wk": wlayout(Wk[:, hs][:, permg]),
                "wv": wlayout(Wv[:, hs]),
                "wo": wo_g,
                "cosp": cos,
                "sinp": sin,
            }
        )
    return in_maps


def kernel(x, Wq, Wk, Wv, Wo, token_position, trace=False, trace_cores=None):
    global LAST_RESULTS
    if trace:
        _ensure_ntff_hook()
    nc = _get_nc()
    in_maps = _shard_inputs(x, Wq, Wk, Wv, Wo, token_position)
    res = run_bass_kernel_spmd(
        nc,
        in_maps,
        core_ids=list(range(N_CORES)),
        trace=trace,
        trace_cores=trace_cores,
    )
    LAST_RESULTS = res
    out = np.empty((B, S, D), dtype=np.float32)
    for core in range(N_CORES):
        b, g = divmod(core, TP)
        shard = np.asarray(res.results[core]["out"], dtype=np.float32)  # [S//TP, D]
        for qs in CHUNK_QTS:
            lo = min(qs)
            rows = len(qs) * 32
            gstart = lo * 128 + g * rows
            out[b, gstart : gstart + rows, :] = shard[lo * 32 : lo * 32 + rows, :]
    return out
